# revision 1
# baseline (speedup 1.0000x reference)
"""BitNet ternary layer on 8 trn2 NeuronCores.

y[b,s,o] = sum_i x[b,s,i] * tq(w)[o,i],  tq(w) = sign(w) * (|w| > 0.7*mean|w|)

Distribution: data-parallel over the batch dim — core c gets x[c] [2048,4096]
and a replicated copy of the full weight [4096,4096]. Each core:
  A) reduces |w| to the global absmean -> threshold t (exact fp32 chain)
  B) casts its x slice to bf16 in DRAM (SWDGE cast DMA)
  C) quantizes w to ternary bf16 ({-1,0,+1} exact in bf16), then streams
     transpose-loads of both operands into SBUF and runs 4096 bf16 matmuls
     (K=128, M=128, N=512) accumulating over 32 k-tiles into PSUM.
Output tiles come back fp32 in the natural [s, o] layout -> host stacks the
8 per-core slices into the [8, 2048, 4096] result.
"""
import copy
import sys

sys.path.insert(0, '/opt/trn_rl_repo')

import numpy as np

import concourse.bass as bass
from concourse import mybir
from concourse.tile import TileContext
from concourse.vector_clock import ScopedClock
from concourse.bass_utils import run_bass_kernel_spmd

# ---------------------------------------------------------------------------
# Workarounds for this container's walrus build rejecting sem-waits attached
# to several instruction structs (CTRL/NoOp/Drain/DMA-transpose): emit the
# TileContext exit waits as standalone wait_ge instructions, and post-process
# the module to hoist every immediate sem-wait onto its own single-wait
# InstEventSemaphore (same engine, same program position -> same semantics).
# ---------------------------------------------------------------------------


def _patched_drain_and_barrier(self, tick_clock, wait_clock):
    probe = self.nc.sync.nop()
    wait_clock.add_sem_waits(probe.ins, ScopedClock({None: tick_clock.global_clock}))
    si = probe.ins.sync_info
    waits = list(si.on_wait) if si is not None else []
    if waits:
        probe.ins.sync_info = mybir.SyncInfo(on_wait=[], on_update=list(si.on_update))
        for w in waits:
            self.nc.sync.wait_ge(bass.SemaphoreHandle(w.ant_name, w.id), w.wait_value)
    self.nc.sync.drain()
    self.nc.all_engine_barrier()
    assert self.sems is not None
    popped = self.nc._tile_sem_poison_stack.pop()
    assert popped is self._sem_poison
    self.nc.clear_and_free_semaphores(list(self.sems.allocated().values()))
    self.nc.all_engine_barrier()


TileContext._drain_and_barrier = _patched_drain_and_barrier

_ctr = [0]


def _hoist_waits(nc):
    new_module = copy.replace(nc.m, functions=[])
    for function in nc.m.functions:
        new_function = copy.replace(function, blocks=[])
        new_function.set_allocations_from_list(function.allocations)
        for block in function.blocks:
            new_insts = []
            for inst in block.instructions:
                si = inst.sync_info
                if si is not None and not isinstance(inst, mybir.InstEventSemaphore):
                    imm = [w for w in si.on_wait if w.wait_reg is None]
                    if imm:
                        reg = [w for w in si.on_wait if w.wait_reg is not None]
                        for w in imm:
                            _ctr[0] += 1
                            ev = mybir.InstEventSemaphore(
                                name=f"HW-{_ctr[0]}", ins=[], outs=[])
                            ev.engine = inst.engine
                            ev.sync_info = mybir.SyncInfo(on_wait=[w], on_update=[])
                            new_insts.append(ev)
                        inst.sync_info = mybir.SyncInfo(
                            on_wait=reg, on_update=list(si.on_update))
                new_insts.append(inst)
            new_block = copy.replace(block, instructions=new_insts)
            new_function.blocks.append(new_block)
        new_module.functions.append(new_function)
    nc.m = new_module
    return nc


# ---------------------------------------------------------------------------
# Problem shapes (hardcoded per spec)
# ---------------------------------------------------------------------------
B = 8            # batch -> one per core
S = 2048         # tokens per core
I = 4096         # in features (contraction)
O = 4096         # out features
P = 128
NK = I // P      # 32 k-tiles
OC = 512         # o-chunk width (one PSUM bank at fp32)
NOC = O // OC    # 8
SH = 1024        # token half kept SBUF-resident as x^T
NH = S // SH     # 2 halves
NSB = SH // P    # 8 s-tiles per half
QF = 2048        # quantize free-dim chunk


def build_program(skip_a=False, skip_quant=False, skip_xpose=False,
                  skip_mm=False, skip_xcast=False, reps=1):
    fp32 = mybir.dt.float32
    bf16 = mybir.dt.bfloat16

    nc = bass.Bass()
    x_in = nc.declare_dram_parameter("x", [S, I], fp32, isOutput=False)
    w_in = nc.declare_dram_parameter("w", [O, I], fp32, isOutput=False)
    y_out = nc.declare_dram_parameter("y", [S, O], fp32, isOutput=True)

    with TileContext(nc) as tc:
        with (
            tc.tile_pool(name="dram", bufs=1, space="DRAM") as dram,
            tc.tile_pool(name="singles", bufs=1) as singles,
            tc.tile_pool(name="psum1", bufs=1, space="PSUM") as psum1,
            tc.tile_pool(name="psum", bufs=6, space="PSUM") as psum_pool,
            tc.tile_pool(name="outsb", bufs=4) as outsb,
        ):
            x16 = dram.tile([S, I], bf16)
            wq_oc = [dram.tile([OC, I], bf16, name=f"wq{oc}") for oc in range(NOC)]
            t_dram = dram.tile([1, 1], fp32)
            partials = singles.tile([P, O // P], fp32)
            part1 = singles.tile([P, 1], fp32)
            ones = singles.tile([P, 1], fp32)
            tval = singles.tile([1, 1], fp32)
            t_b = singles.tile([P, 1], fp32)
            nt_b = singles.tile([P, 1], fp32)

            for rep in range(reps):
                # ---- Phase B: cast x fp32 -> bf16 in DRAM (SWDGE cast) ----
                if not skip_xcast:
                    for j in range(16):
                        nc.gpsimd.dma_start(
                            out=x16[j * P:(j + 1) * P, :],
                            in_=x_in[j * P:(j + 1) * P, :])

                # ---- Phase A: threshold t = 0.7 * mean|w| ----
                if skip_a:
                    nc.vector.memset(t_b[:], 0.5585)
                    nc.vector.memset(nt_b[:], -0.5585)
                if not skip_a:
                    with tc.tile_pool(name="pha", bufs=3) as pha:
                        for j in range(O // P):
                            wa = pha.tile([P, I], fp32)
                            nc.sync.dma_start(
                                out=wa[:], in_=w_in[j * P:(j + 1) * P, :])
                            nc.vector.tensor_reduce(
                                partials[:, j:j + 1], wa[:],
                                axis=mybir.AxisListType.X,
                                op=mybir.AluOpType.add,
                                apply_absolute_value=True)
                    nc.vector.tensor_reduce(
                        part1[:], partials[:], axis=mybir.AxisListType.X,
                        op=mybir.AluOpType.add)
                    nc.vector.memset(ones[:], 1.0)
                    tsum = psum1.tile([1, 1], fp32)
                    nc.tensor.matmul(tsum[:], lhsT=part1[:], rhs=ones[:],
                                     start=True, stop=True)
                    nc.scalar.activation(tval[:], tsum[:],
                                         mybir.ActivationFunctionType.Copy,
                                         scale=0.7 / float(O * I))
                    nc.sync.dma_start(out=t_dram[:], in_=tval[:])
                    t_bcast_ap = bass.AP(
                        tensor=t_dram.tensor, offset=t_dram.offset,
                        ap=[[0, P], [1, 1]])
                    nc.gpsimd.dma_start(out=t_b[:], in_=t_bcast_ap)
                    nc.vector.tensor_scalar_mul(nt_b[:], t_b[:], -1.0)

                # ---- Phase C: quantize + matmul pipeline ----
                with (
                    tc.tile_pool(name="quant", bufs=2) as quant,
                    tc.tile_pool(name="xT_pool", bufs=1) as xT_pool,
                    tc.tile_pool(name="wqT_pool", bufs=2) as wqT_pool,
                ):
                    xT = xT_pool.tile([P, NK, SH], bf16)
                    for h in range(NH):
                        if not skip_xpose:
                            for k in range(NK):
                                nc.sync.dma_start(
                                    out=xT[:, k, :],
                                    in_=x16[h * SH:(h + 1) * SH,
                                            k * P:(k + 1) * P],
                                    transpose=True)
                        for oc in range(NOC):
                            if h == 0 and not skip_quant:
                                for mb in range(OC // P):
                                    r0 = oc * OC + mb * P
                                    for cc in range(I // QF):
                                        c0 = cc * QF
                                        wb = quant.tile([P, QF], fp32,
                                                        tag="wb")
                                        nc.sync.dma_start(
                                            out=wb[:],
                                            in_=w_in[r0:r0 + P, c0:c0 + QF])
                                        pt = quant.tile([P, QF], bf16,
                                                        tag="pt")
                                        nt = quant.tile([P, QF], bf16,
                                                        tag="nt")
                                        qt = quant.tile([P, QF], bf16,
                                                        tag="qt")
                                        nc.vector.tensor_scalar(
                                            pt[:], wb[:], t_b[:], None,
                                            op0=mybir.AluOpType.is_gt)
                                        nc.vector.tensor_scalar(
                                            nt[:], wb[:], nt_b[:], None,
                                            op0=mybir.AluOpType.is_lt)
                                        nc.vector.tensor_sub(qt[:], pt[:],
                                                             nt[:])
                                        nc.scalar.dma_start(
                                            out=wq_oc[oc][
                                                mb * P:(mb + 1) * P,
                                                c0:c0 + QF],
                                            in_=qt[:])
                            wqT = wqT_pool.tile([P, NK, OC], bf16)
                            if not skip_xpose:
                                for k in range(NK):
                                    nc.sync.dma_start(
                                        out=wqT[:, k, :],
                                        in_=wq_oc[oc][:, k * P:(k + 1) * P],
                                        transpose=True)
                            if not skip_mm:
                                for s in range(NSB):
                                    ps = psum_pool.tile([P, OC], fp32)
                                    for k in range(NK):
                                        nc.tensor.matmul(
                                            ps[:],
                                            lhsT=xT[:, k, s * P:(s + 1) * P],
                                            rhs=wqT[:, k, :],
                                            start=(k == 0),
                                            stop=(k == NK - 1))
                                    ob = outsb.tile([P, OC], fp32)
                                    nc.scalar.activation(
                                        ob[:], ps[:],
                                        mybir.ActivationFunctionType.Copy)
                                    nc.scalar.dma_start(
                                        out=y_out[
                                            h * SH + s * P:
                                            h * SH + (s + 1) * P,
                                            oc * OC:(oc + 1) * OC],
                                        in_=ob[:])
                if reps > 1:
                    tc.strict_bb_all_engine_barrier()

    _hoist_waits(nc)
    return nc


_program_cache = None


def _get_program():
    global _program_cache
    if _program_cache is None:
        _program_cache = build_program()
    return _program_cache


def run(x, weight, trace=False):
    x = np.asarray(x, dtype=np.float32)
    weight = np.ascontiguousarray(np.asarray(weight, dtype=np.float32))
    assert x.shape == (B, S, I), x.shape
    assert weight.shape == (O, I), weight.shape
    nc = _get_program()
    in_maps = [{"x": np.ascontiguousarray(x[c]), "w": weight} for c in range(B)]
    res = run_bass_kernel_spmd(nc, in_maps, list(range(B)), trace=trace)
    y = np.stack([res.results[c]["y"] for c in range(B)], axis=0)
    return y, res


def kernel(x, weight):
    y, _ = run(x, weight)
    return y

